# revision 18
# baseline (speedup 1.0000x reference)
"""Causal linear attention (ELU+1 feature map) on 8 trn2 NeuronCores.

Sharding: core i handles batch b=i//2, sequence half h=i%2 (T=2048 -> 1024
tokens/core).  Second-half cores recompute the first half's running state
S0 = sum_tau phi(k_tau) [v_tau, 1]  (128x129, col 128 = z) from k/v of the
first half; first-half cores get zeroed aux inputs so their S0 == 0.

Math per core (chunk C=128, 8 own chunks + 8 "pre" state-only chunks):
  phi(y) = min(exp(y), max(y+1, 1))            (== ELU(y)+1 exactly)
  A^T_c = K_c Q_c^T ; mask (tau<=t) fused into PSUM->SBUF bf16 copy
  O_c = Amask^T.T @ [V_c, 1] + Q_c @ (Se + So)  (den accumulates in col 128)
  Se/So: parity-split state accumulators (halves the snapshot chain)
  out_c = O_c[:, :128] * (1 / O_c[:, 128])
"""

import numpy as np

B, T, D, DV = 4, 2048, 128, 128
H = T // 2          # tokens per core
C = 128             # chunk
NCH = H // C        # chunks per half
NCORES = 8
VW = DV + 1

# bf16 pack columns, need-ordered:
# [WTb|bias|btrow | kTp | vp | mask4|ident | qT1 kT1 | v | qT2 kT2]
OFF_WTB = 0
OFF_BIAS = OFF_WTB + D           # 1 col
OFF_BTROW = OFF_BIAS + 1         # bias as a row in partition 0 (128 cols)
OFF_KTP = OFF_BTROW + C          # [D, H] pre-half k^T
OFF_VP = OFF_KTP + H             # [C, NCH*VW] pre-half v (chunk-major)
OFF_MASK4 = OFF_VP + NCH * VW    # causal mask tiled 4x (512)
OFF_ID = OFF_MASK4 + 4 * C
OFF_QKT1 = OFF_ID + C            # qT cols 0:512 | kT cols 0:512
OFF_V = OFF_QKT1 + H             # own v
OFF_QKT2 = OFF_V + NCH * VW      # qT cols 512:1024 | kT cols 512:1024
B16_COLS = OFF_QKT2 + H

# NOTE: Pool/GPSIMD cannot access PSUM on hardware, and only supports
# tensor_scalar-class elementwise ops -- Pool gets min(e,1) in "v2a" phi
# slices plus DMA descriptor generation; everything else is Act/DVE.
# phi slice modes (order: tok0, tok1, q0, k0, q1, k1):
#   v1:  e=exp (Act); r1=max(y+1,1) (DVE tensor_scalar); phi=min(e,r1) (DVE)
#   v2a: e=exp (Act); r=relu (Act); ec=min(e,1) (Pool); phi=ec+r (DVE)
CFG = {
    "phi_mode": ("v1", "v1", "v1", "v2a", "v1", "v2a"),
    "ktok_eng": ("act", "act"),    # per chunk-quad: act | dve
    "snap_eng": "alt",             # dve | act | alt
    "scale_eng": "dve",            # act | dve | alt (div: invalid ISA)
    "dma_plan": (("sp", 0), ("pool", 1), ("sp", 2), ("pool", 3),
                 ("sp", 4), ("pool", 5), ("sp", 6)),
    "wrk_bufs": 2,
    "npar": 2,
    "pso_bufs": 2,
    "psa_bufs": 2,
    "out_pieces": 4,
    "out_mode": "dma",             # dma | trig
    "out_bf16": True,
}

_cache = {}


def _build(cfg=None):
    import concourse.bacc as bacc
    import concourse.tile as tile
    from concourse import mybir
    from bass_rust import add_dep_helper

    cfg = dict(CFG, **(cfg or {}))
    F32 = mybir.dt.float32
    BF16 = mybir.dt.bfloat16
    AF = mybir.ActivationFunctionType
    MUL = mybir.AluOpType.mult
    ADD = mybir.AluOpType.add
    MAX = mybir.AluOpType.max
    MIN = mybir.AluOpType.min
    ODT = BF16 if cfg["out_bf16"] else F32

    nc = bacc.Bacc(None, target_bir_lowering=False, debug=False,
                   num_devices=NCORES)

    bin_ = nc.declare_dram_parameter("bin", [D, B16_COLS], BF16,
                                     isOutput=False)
    out = nc.declare_dram_parameter("out", [C, NCH * DV], ODT, isOutput=True)

    def eng(which):
        return {"dve": nc.vector, "act": nc.scalar, "pool": nc.gpsimd}[which]

    def copy_eng(which, dst, srcp):
        if which == "act":
            nc.scalar.activation(dst, srcp, AF.Copy)
        else:
            eng(which).tensor_copy(dst, srcp)

    with tile.TileContext(nc) as tc:
        with (
            tc.tile_pool(name="cst", bufs=1) as cst,
            tc.tile_pool(name="io", bufs=1) as io,
            tc.tile_pool(name="phi", bufs=1) as phip,
            tc.tile_pool(name="am", bufs=NCH // 4) as amp,
            tc.tile_pool(name="wrk", bufs=cfg["wrk_bufs"]) as wrk,
            tc.tile_pool(name="ps_pre", bufs=2, space="PSUM") as ps_pre,
            tc.tile_pool(name="ps_s", bufs=cfg["npar"], space="PSUM") as ps_s,
            tc.tile_pool(name="ps_a", bufs=cfg["psa_bufs"],
                         space="PSUM") as ps_a,
            tc.tile_pool(name="ps_o", bufs=cfg["pso_bufs"],
                         space="PSUM") as ps_o,
        ):
            # ---- warm the ACT exp table while DMAs run ----
            s_warm = cst.tile([D, 1], F32)
            nc.vector.memset(s_warm, 0.0)
            s_warm2 = cst.tile([D, 1], BF16)
            nc.scalar.activation(s_warm2, s_warm, AF.Exp)

            # ---- input loads: need-ordered pieces on hwdge (sp) + swdge ----
            s_b16 = io.tile([D, B16_COLS], BF16)
            s_ones = cst.tile([1, C], BF16)
            nc.vector.memset(s_ones, 1.0)
            pieces = [
                (0, OFF_KTP),                 # 0: WTb|bias|btrow
                (OFF_KTP, OFF_KTP + 512),     # 1: kTp half 0
                (OFF_KTP + 512, OFF_MASK4),   # 2: kTp half 1 + vp
                (OFF_MASK4, OFF_QKT1),        # 3: mask4|ident
                (OFF_QKT1, OFF_V),            # 4: qT1|kT1
                (OFF_V, OFF_QKT2),            # 5: v
                (OFF_QKT2, B16_COLS),         # 6: qT2|kT2
            ]
            for which, idx in cfg["dma_plan"]:
                a, b = pieces[idx]
                e = nc.sync if which == "sp" else (
                    nc.gpsimd if which == "pool" else eng(which))
                e.dma_start(out=s_b16[:, a:b], in_=bin_[:, a:b])

            s_bias = s_b16[:, OFF_BIAS:OFF_BIAS + 1]
            s_biasp1 = cst.tile([D, 1], F32)
            nc.vector.tensor_scalar_add(s_biasp1, s_bias, 1.0)
            sWTb = s_b16[:, OFF_WTB:OFF_WTB + D]
            s_mask4 = s_b16[:, OFF_MASK4:OFF_MASK4 + 4 * C]
            s_ident = s_b16[:, OFF_ID:OFF_ID + C]

            # bias row [1, D] packed in partition 0 of the consts piece
            s_btrow = s_b16[0:1, OFF_BTROW:OFF_BTROW + D]

            def vsl(c):
                return s_b16[:, OFF_V + VW * c:OFF_V + VW * (c + 1)]

            def vpsl(c):
                return s_b16[:, OFF_VP + VW * c:OFF_VP + VW * (c + 1)]

            # feature-major phi storage: [Qj0 | Kj0 | Qj1 | Kj1] blocks of 512
            phi_f = phip.tile([D, 2 * H], BF16)
            e_f = phip.tile([D, 2 * H], BF16)
            r_f = phip.tile([D, 2 * H], BF16)
            ec_f = phip.tile([D, 2 * H], BF16)

            def qsl(c):
                j, cc = divmod(c, 4)
                return phi_f[:, 1024 * j + C * cc:1024 * j + C * (cc + 1)]

            def ksl(c):
                j, cc = divmod(c, 4)
                base = 1024 * j + 512
                return phi_f[:, base + C * cc:base + C * (cc + 1)]

            # parity-split state accumulators [D, DV+1]
            NPAR = cfg["npar"]
            Sp = []
            for i in range(NPAR):
                S_i = ps_s.tile([D, DV + 1], F32, tag="s")
                Sp.append(S_i)
            started = [False] * NPAR
            s_first = [None] * NPAR

            # ---- token-major phi for K_pre (state recompute path) ----
            phi_t = phip.tile([C, H], BF16)
            e_t = phip.tile([C, H], BF16)
            r_t = phip.tile([C, H], BF16)
            ec_t = phip.tile([C, H], BF16)

            def tok_block(j):
                pst = ps_pre.tile([C, 512], F32, tag="pre")
                prev = None
                for cc in range(4):
                    c = 4 * j + cc
                    csl = slice(C * cc, C * (cc + 1))
                    mm_b = nc.tensor.matmul(pst[:, csl], s_ones, s_btrow,
                                            start=True, stop=False)
                    if prev is not None:
                        add_dep_helper(mm_b.ins, prev.ins, sync=False,
                                       reason="psum group order")
                    mm_c = nc.tensor.matmul(
                        pst[:, csl],
                        s_b16[:, OFF_KTP + C * c:OFF_KTP + C * (c + 1)],
                        sWTb, start=False, stop=(cc == 3))
                    add_dep_helper(mm_c.ins, mm_b.ins, sync=False,
                                   reason="psum group order")
                    prev = mm_c
                sl = slice(512 * j, 512 * (j + 1))
                nc.scalar.activation(e_t[:, sl], pst, AF.Exp)
                if cfg["phi_mode"][j] == "v1":
                    nc.vector.tensor_scalar(out=r_t[:, sl], in0=pst,
                                            scalar1=1.0, scalar2=1.0,
                                            op0=ADD, op1=MAX)
                    nc.vector.tensor_tensor(out=phi_t[:, sl],
                                            in0=e_t[:, sl],
                                            in1=r_t[:, sl], op=MIN)
                else:
                    nc.scalar.activation(r_t[:, sl], pst, AF.Relu)
                    nc.gpsimd.tensor_scalar_min(ec_t[:, sl], e_t[:, sl], 1.0)
                    nc.vector.tensor_tensor(out=phi_t[:, sl],
                                            in0=ec_t[:, sl],
                                            in1=r_t[:, sl], op=ADD)
                # pre-half state contributions (zeros on half-0 cores)
                for cc in range(4):
                    c = 4 * j + cc
                    p = c % NPAR
                    mm_s = nc.tensor.matmul(Sp[p],
                                            phi_t[:, C * c:C * (c + 1)],
                                            vpsl(c),
                                            start=(not started[p]),
                                            stop=False, skip_group_check=True)
                    if started[p]:
                        add_dep_helper(mm_s.ins, s_first[p].ins, sync=False,
                                       reason="psum group order")
                    s_first[p] = mm_s
                    started[p] = True

            # ---- feature-major phi for own q,k: 512-col slices ----
            def phi_slice(j, i):
                # j: token half, i: 0=q 1=k
                off = (OFF_QKT1 if j == 0 else OFF_QKT2) + 512 * i
                pre = ps_pre.tile([D, 512], F32, tag="pre")
                nc.tensor.matmul(pre, sWTb, s_b16[:, off:off + 512],
                                 start=True, stop=True)
                sl = slice(1024 * j + 512 * i, 1024 * j + 512 * (i + 1))
                nc.scalar.activation(e_f[:, sl], pre, AF.Exp,
                                     bias=s_bias, scale=1.0)
                if cfg["phi_mode"][2 + 2 * j + i] == "v1":
                    nc.vector.tensor_scalar(out=r_f[:, sl], in0=pre,
                                            scalar1=s_biasp1, scalar2=1.0,
                                            op0=ADD, op1=MAX)
                    nc.vector.tensor_tensor(out=phi_f[:, sl],
                                            in0=e_f[:, sl],
                                            in1=r_f[:, sl], op=MIN)
                else:
                    nc.scalar.activation(r_f[:, sl], pre, AF.Relu,
                                         bias=s_bias, scale=1.0)
                    nc.gpsimd.tensor_scalar_min(ec_f[:, sl], e_f[:, sl], 1.0)
                    nc.vector.tensor_tensor(out=phi_f[:, sl],
                                            in0=ec_f[:, sl],
                                            in1=r_f[:, sl], op=ADD)

            ktok = phip.tile([C, H], BF16)
            Am = [None] * (NCH // 4)

            def prep_quad(qc):
                cs = range(4 * qc, 4 * qc + 4)
                trp = ps_a.tile([C, 4 * C], BF16, tag="a")
                prev = None
                for i, c in enumerate(cs):
                    t = nc.tensor.transpose(trp[:, C * i:C * (i + 1)],
                                            ksl(c), s_ident)
                    if prev is not None:
                        add_dep_helper(t.ins, prev.ins, sync=False,
                                       reason="psum group order")
                    prev = t
                copy_eng(cfg["ktok_eng"][qc],
                         ktok[:, C * (4 * qc):C * (4 * qc + 4)], trp)
                Ap = ps_a.tile([C, 4 * C], F32, tag="a")
                prev = None
                for i, c in enumerate(cs):
                    a = nc.tensor.matmul(Ap[:, C * i:C * (i + 1)],
                                         ksl(c), qsl(c),
                                         start=True, stop=True)
                    if prev is not None:
                        add_dep_helper(a.ins, prev.ins, sync=False,
                                       reason="psum group order")
                    prev = a
                am_p = amp.tile([C, 4 * C], BF16, tag="am")
                Am[qc] = am_p
                nc.vector.tensor_tensor(out=am_p, in0=Ap, in1=s_mask4, op=MUL)

            outstage = phip.tile([C, NCH * DV], ODT)
            snaps = [None] * NPAR

            # trigger-mode output: scatter rows 2p+piece of out (viewed as
            # [-1, 512]); descriptors prepared early on the idle Pool engine,
            # fired by trigger_dma when each half of outstage is complete.
            trig = cfg["out_mode"] == "trig"
            if trig:
                I16 = mybir.dt.int16
                s_idx = cst.tile([16, 16], I16)
                out_sems = []

                def out_prep(piece):
                    nc.gpsimd.iota(s_idx[:, 8 * piece:8 * (piece + 1)],
                                   [[32, 8]], base=piece,
                                   channel_multiplier=2)
                    sem = nc.alloc_semaphore(f"out_dma{piece}")
                    out_sems.append(sem)
                    nc.gpsimd.dma_scatter_add(
                        out[:, :],
                        outstage[:, 512 * piece:512 * (piece + 1)],
                        s_idx[:, 8 * piece:8 * (piece + 1)],
                        128, 128, 512,
                        prepare_only=True, sem=sem)

                out_prep(0)

            def run_chunk(c):
                pl = tuple(range(NPAR)) if c == 0 else ((c - 1) % NPAR,)
                for p in pl:
                    snp = wrk.tile([D, DV + 1], BF16, tag=f"snap{p}")
                    snaps[p] = snp
                    se_ = cfg["snap_eng"]
                    if se_ == "alt":
                        se_ = "dve" if c % 2 == 0 else "act"
                    copy_eng(se_, snaps[p], Sp[p])

                amsl = Am[c // 4][:, (c % 4) * C:(c % 4 + 1) * C]
                O = ps_o.tile([C, DV + 1], F32, tag="o")
                prev_o = nc.tensor.matmul(O, amsl, vsl(c), start=True,
                                          stop=False)
                for qi, sn in enumerate(snaps):
                    mm_q = nc.tensor.matmul(O, qsl(c), sn, start=False,
                                            stop=(qi == len(snaps) - 1))
                    add_dep_helper(mm_q.ins, prev_o.ins, sync=False,
                                   reason="psum group order")
                    prev_o = mm_q

                mm_su = nc.tensor.matmul(Sp[c % NPAR],
                                         ktok[:, C * c:C * (c + 1)], vsl(c),
                                         start=False, stop=(c >= NCH - NPAR),
                                         skip_group_check=True)
                add_dep_helper(mm_su.ins, s_first[c % NPAR].ins, sync=False,
                               reason="psum group order")
                s_first[c % NPAR] = mm_su

                se = cfg["scale_eng"]
                if se == "div":
                    nc.vector.tensor_scalar(
                        out=outstage[:, DV * c:DV * (c + 1)], in0=O[:, 0:DV],
                        scalar1=O[:, DV:DV + 1], scalar2=None,
                        op0=mybir.AluOpType.divide)
                else:
                    rec = wrk.tile([C, 1], F32, tag="rec")
                    nc.vector.reciprocal(rec, O[:, DV:DV + 1])
                    which = ("act" if se == "act" else
                             "dve" if se == "dve" else
                             ("act" if c % 2 == 0 else "dve"))
                    if which == "act":
                        nc.scalar.activation(outstage[:, DV * c:DV * (c + 1)],
                                             O[:, 0:DV], AF.Copy, bias=0.0,
                                             scale=rec)
                    else:
                        nc.vector.tensor_scalar_mul(
                            outstage[:, DV * c:DV * (c + 1)], O[:, 0:DV], rec)
                if trig:
                    if c == NCH // 2 - 1:
                        nc.gpsimd.trigger_dma(count=None)
                        out_prep(1)
                    elif c == NCH - 1:
                        nc.gpsimd.trigger_dma(count=None)
                else:
                    np_ = cfg["out_pieces"]
                    if np_ == 2:
                        if c in (NCH // 2 - 1, NCH - 1):
                            a = 0 if c == NCH // 2 - 1 else NCH * DV // 2
                            b_ = (NCH * DV // 2 if c == NCH // 2 - 1
                                  else NCH * DV)
                            nc.sync.dma_start(out=out[:, a:b_],
                                              in_=outstage[:, a:b_])
                    elif np_ == 8:
                        nc.sync.dma_start(
                            out=out[:, DV * c:DV * (c + 1)],
                            in_=outstage[:, DV * c:DV * (c + 1)])
                    else:
                        if c % 2 == 1:
                            nc.sync.dma_start(
                                out=out[:, DV * (c - 1):DV * (c + 1)],
                                in_=outstage[:, DV * (c - 1):DV * (c + 1)])

            # ---- schedule ----
            tok_block(0)
            tok_block(1)
            phi_slice(0, 0)
            phi_slice(0, 1)
            prep_quad(0)
            run_chunk(0)
            run_chunk(1)
            phi_slice(1, 0)
            phi_slice(1, 1)
            run_chunk(2)
            run_chunk(3)
            prep_quad(1)
            for c in range(4, NCH):
                run_chunk(c)

    nc.compile()
    return nc


def _get_nc():
    if "nc" not in _cache:
        _cache["nc"] = _build()
    return _cache["nc"]


def _pack_inputs(q, k, v, W_phi, b_phi):
    import ml_dtypes
    bf16 = ml_dtypes.bfloat16

    WT = np.ascontiguousarray(W_phi.T)                    # [d, e]
    maskm = np.triu(np.ones((C, C), np.float32))          # keep tau <= t
    ident = np.eye(C, dtype=np.float32)

    def aug(vh):  # [H, DV] -> [C, NCH*(DV+1)] partition-major with ones col
        a = np.concatenate([vh, np.ones((H, 1), np.float32)], axis=1)
        return a.reshape(NCH, C, VW).transpose(1, 0, 2).reshape(C, NCH * VW)

    zeros_vp = np.zeros((C, NCH * VW), np.float32)
    zeros_ktp = np.zeros((D, H), np.float32)

    in_maps = []
    for core in range(NCORES):
        b_idx, half = divmod(core, 2)
        sl = slice(half * H, (half + 1) * H)
        b16 = np.empty((D, B16_COLS), np.float32)
        b16[:, OFF_WTB:OFF_WTB + D] = WT
        for mi in range(4):
            b16[:, OFF_MASK4 + mi * C:OFF_MASK4 + (mi + 1) * C] = maskm
        b16[:, OFF_ID:OFF_ID + C] = ident
        b16[:, OFF_BIAS] = b_phi
        b16[:, OFF_BTROW:OFF_BTROW + C] = 0.0
        b16[0, OFF_BTROW:OFF_BTROW + C] = b_phi
        qh = q[b_idx, sl].T
        kh = k[b_idx, sl].T
        b16[:, OFF_QKT1:OFF_QKT1 + 512] = qh[:, 0:512]
        b16[:, OFF_QKT1 + 512:OFF_QKT1 + 1024] = kh[:, 0:512]
        b16[:, OFF_QKT2:OFF_QKT2 + 512] = qh[:, 512:1024]
        b16[:, OFF_QKT2 + 512:OFF_QKT2 + 1024] = kh[:, 512:1024]
        if half == 1:
            b16[:, OFF_KTP:OFF_KTP + H] = k[b_idx, 0:H].T
            b16[:, OFF_VP:OFF_VP + NCH * VW] = aug(v[b_idx, 0:H])
        else:
            b16[:, OFF_KTP:OFF_KTP + H] = zeros_ktp
            b16[:, OFF_VP:OFF_VP + NCH * VW] = zeros_vp
        b16[:, OFF_V:OFF_V + NCH * VW] = aug(v[b_idx, sl])
        in_maps.append({"bin": b16.astype(bf16)})
    return in_maps


def kernel(q, k, v, W_phi, b_phi):
    from concourse.bass_utils import run_bass_kernel_spmd

    q = np.asarray(q, np.float32)
    k = np.asarray(k, np.float32)
    v = np.asarray(v, np.float32)
    W_phi = np.asarray(W_phi, np.float32)
    b_phi = np.asarray(b_phi, np.float32)

    in_maps = _pack_inputs(q, k, v, W_phi, b_phi)
    nc = _get_nc()
    res = run_bass_kernel_spmd(nc, in_maps, list(range(NCORES)))

    out = np.empty((B, T, DV), np.float32)
    for core in range(NCORES):
        b_idx, half = divmod(core, 2)
        o = np.asarray(res.results[core]["out"],
                       dtype=np.float32)                  # [C, NCH*DV]
        o = o.reshape(C, NCH, DV).transpose(1, 0, 2).reshape(H, DV)
        out[b_idx, half * H:(half + 1) * H] = o
    return out
